# revision 30
# baseline (speedup 1.0000x reference)
"""Causal attention kernel for Trainium2 (Bass/Tile), 8-core data-parallel.

Problem: x [8, 2048, 1024] f32; W_query/W_key/W_value [1024, 1024] f32.
    q = x @ Wq; k = x @ Wk; v = x @ Wv       (per batch element)
    out = softmax(causal(q k^T) / 32) @ v

Sharding: batch dim (8) across the 8 NeuronCores, one batch element per
core; each core runs the identical single-core program on its slice.

Algorithm (per core): scores are computed as  S = x (Wq Wk^T) x^T,
which removes one full [2048,1024]x[1024,1024] projection pass: with
M = Wq Wk^T precomputed ([1024,1024], 64K PE-cycles incl. weight
transposes vs 128K for the k-pass), the "keys" operand of the score
matmuls is x^T itself, which is already resident for the projections.

Phase 1:
  Wq, Wk panels stream in; PE-transpose to WqT/WkT while accumulating
  the first 3 row-chunks of M in PSUM (pipelined against the DMA).
  Remaining 5 M chunks follow, then x streams in and is PE-transposed
  to xT [d_in, tok] (full 2048 tokens resident).
  qT = M^T x^T pass: chunks j=0,1 (tokens 0..1023) are written straight
  into the phase-2 SBUF tiles (no DRAM roundtrip); j=2,3 spill to DRAM.
  v = x Wv pass in natural [tok, d] layout (xT stationary, Wv moving).
Phase 2 (per 512-query chunk j, 128-query block i, descending il):
  S tiles via lhsT=qT block, rhs=xT chunks (causal: skip m > n chunks)
  additive causal mask on the diagonal chunk
  expP = exp(S/32) with fused row-sum (denominator partials)
  PE-transpose expP 128x128 blocks -> PT[m, n]
  O[n, d] += PT^T v  accumulated in PSUM over all valid m blocks
  O * (1/denominator) -> DMA out

All matmuls run in float32r (full PE rate for moving dim >= 256).
"""

import os

import numpy as np

# Defensive: recover wedged cores at NRT/PJRT init (no-op on healthy devices).
os.environ.setdefault("NEURON_RT_RESET_CORES", "1")

import concourse.tile as tile
import concourse.mybir as mybir
from concourse import bacc, bass_utils
from concourse.masks import make_identity

F32 = mybir.dt.float32
F32R = mybir.dt.float32r
BF16 = mybir.dt.bfloat16
EXP = mybir.ActivationFunctionType.Exp
AXX = mybir.AxisListType.X

NTOK = 2048      # tokens per batch element (= per core)
D = 1024         # d_in = d_out
P = 128          # partitions
DC = D // P      # 8 d-chunks
NBLK = NTOK // P     # 16 token blocks
NJ = NTOK // 512     # 4 query chunks of 512
NEG = -1.0e9
SCALE = 1.0 / 32.0   # 1/sqrt(D)
MA = 3           # M row-chunks accumulated during the weight stream


def build_program():
    nc = bacc.Bacc("TRN2", target_bir_lowering=False, debug=False,
                   num_devices=8)
    x = nc.dram_tensor("x", [NTOK, D], F32, kind="ExternalInput").ap()
    wq = nc.dram_tensor("W_query", [D, D], F32, kind="ExternalInput").ap()
    wk = nc.dram_tensor("W_key", [D, D], F32, kind="ExternalInput").ap()
    wv = nc.dram_tensor("W_value", [D, D], F32, kind="ExternalInput").ap()
    out = nc.dram_tensor("out", [NTOK, D], F32, kind="ExternalOutput").ap()

    with tile.TileContext(nc) as tc:
        _emit(nc, tc, x, wq, wk, wv, out)
    nc.compile()
    return nc


def _emit(nc, tc, x, wq, wk, wv, out):
    const = tc.alloc_tile_pool(name="const", bufs=1)

    # constants: identity for PE transpose, additive causal mask strip
    id32 = const.tile([P, P], F32, tag="id32")
    make_identity(nc, id32)
    id32r = const.tile([P, P], F32R, tag="id32r")
    nc.vector.tensor_copy(id32r, id32)
    id16 = const.tile([P, P], BF16, tag="id16")
    nc.vector.tensor_copy(id16, id32)
    # maskA = [0 x384 | causal(128) | NEG x384]; slice width 512 starting at
    # (3-il)*128 puts the causal block at in-chunk block position il.
    maskA = const.tile([P, 896], F32, tag="maskA")
    nc.vector.memset(maskA[:, 0:384], 0.0)
    nc.vector.memset(maskA[:, 384:512], 0.0)
    nc.gpsimd.affine_select(
        out=maskA[:, 384:512], in_=maskA[:, 384:512],
        compare_op=mybir.AluOpType.is_ge, fill=NEG, base=0,
        pattern=[[-1, P]], channel_multiplier=1)
    nc.vector.memset(maskA[:, 512:896], NEG)

    def copy_balanced(sel, out_ap, in_ap):
        # split PSUM->SBUF copy traffic across ACT and DVE
        if sel % 2 == 0:
            nc.scalar.copy(out_ap, in_ap)
        else:
            nc.vector.tensor_copy(out_ap, in_ap)

    # ---------------- phase 1a: weight transposes + M = Wq Wk^T ----------
    ps2 = tc.alloc_tile_pool(name="ps2", bufs=2, space="PSUM")   # 2 banks
    psM = tc.alloc_tile_pool(name="psM", bufs=3, space="PSUM")   # 6 banks
    msbp = tc.alloc_tile_pool(name="msbp", bufs=1, side="right")
    xnat = tc.alloc_tile_pool(name="xnat", bufs=32, side="right")
    xTp = tc.alloc_tile_pool(name="xTp", bufs=1)
    wqTp = tc.alloc_tile_pool(name="wqTp", bufs=1)
    wkTp = tc.alloc_tile_pool(name="wkTp", bufs=1)
    wstage = tc.alloc_tile_pool(name="wstage", bufs=6)

    xT = xTp.tile([P, DC, NTOK], BF16, tag="xT")  # [din%128, dinchunk, tok]
    wqT = wqTp.tile([P, DC, D], BF16, tag="wqT")   # [dout%128, oc, din]
    wkT = wkTp.tile([P, DC, D], BF16, tag="wkT")
    msb = msbp.tile([P, DC, D], BF16, tag="msb")   # M: [a%128, a-chunk, b]

    # DMA order: two x tiles (to fill the PE pipeline at t=0), then all W
    # panels (Wq/Wk alternating), then the rest of the x stream.
    xts = []

    def dma_x(tb, g):
        xt = xnat.tile([P, 512], F32R, tag="xt")
        nc.sync.dma_start(
            out=xt,
            in_=x[tb * P:(tb + 1) * P, g * 512:(g + 1) * 512].bitcast(F32R))
        xts.append(xt)

    def emit_xT(t):
        for g in range(2):
            xt = xts[t * 2 + g]
            trp = ps2.tile([P, 512], F32R, tag="ps2")
            for b4 in range(4):
                nc.tensor.transpose(
                    trp[:, b4 * P:(b4 + 1) * P],
                    xt[:, b4 * P:(b4 + 1) * P], id32r)
            copy_balanced(
                g, xT[:, g * 4:(g + 1) * 4, t * P:(t + 1) * P],
                trp.rearrange("p (b f) -> p b f", b=4))

    dma_x(0, 0)
    dma_x(0, 1)
    pans = []
    for oc in range(DC):
        for w_ap in (wq, wk):
            pan = wstage.tile([P, DC, P], F32R, tag="pan")
            nc.sync.dma_start(
                out=pan,
                in_=w_ap[:, oc * P:(oc + 1) * P]
                .rearrange("(c p) f -> p c f", p=P).bitcast(F32R))
            pans.append(pan)
    for tb in range(NBLK):
        for g in range(2):
            if tb == 0:
                continue
            dma_x(tb, g)
    emit_xT(0)  # PE starts here while the first W panel is in flight

    # W-T interleaved with the first MA chunks of M (PSUM-resident
    # accumulators, one matmul chain per chunk across all oc panels).
    Mps = [psM.tile([P, D], F32, tag="psM", name=f"Mps{a}") for a in range(MA)]
    for oc in range(DC):
        for iw, wT in ((0, wqT), (1, wkT)):
            pan = pans[oc * 2 + iw]
            for half in range(2):
                trp = ps2.tile([P, 512], F32R, tag="ps2")
                for ic4 in range(4):
                    ic = half * 4 + ic4
                    nc.tensor.transpose(
                        trp[:, ic4 * P:(ic4 + 1) * P], pan[:, ic, :], id32r)
                copy_balanced(oc + iw + half,
                              wT[:, oc, half * 512:(half + 1) * 512], trp)
        for a in range(MA):
            for h in range(2):  # PSUM bank limit: <=512 f32 out per matmul
                nc.tensor.matmul(
                    Mps[a][:, h * 512:(h + 1) * 512],
                    wqT[:, oc, a * P:(a + 1) * P],
                    wkT[:, oc, h * 512:(h + 1) * 512],
                    start=(oc == 0), stop=(oc == DC - 1))
    for a in range(MA):
        copy_balanced(a, msb[:, a, :], Mps[a])
    for a in range(MA, DC):
        ps = psM.tile([P, D], F32, tag="psM")
        for oc in range(DC):
            for h in range(2):
                nc.tensor.matmul(
                    ps[:, h * 512:(h + 1) * 512],
                    wqT[:, oc, a * P:(a + 1) * P],
                    wkT[:, oc, h * 512:(h + 1) * 512],
                    start=(oc == 0), stop=(oc == DC - 1))
        copy_balanced(a, msb[:, a, :], ps)

    wstage.release()
    wkTp.release()
    wqTp.release()
    psM.release()

    # ---------------- phase 1b: x transpose (t=0 was done up front) ------
    ps4 = tc.alloc_tile_pool(name="ps4", bufs=4, space="PSUM")
    for t in range(1, NBLK):
        emit_xT(t)
    xnat.release()

    # phase-2 / v-pass pools come up now so their first loads can issue on
    # the ACT queue early.
    p2q = tc.alloc_tile_pool(name="p2q", bufs=4)
    vp = tc.alloc_tile_pool(name="vp", bufs=1)
    wvstage = tc.alloc_tile_pool(name="wvstage", bufs=2)
    wvpool = tc.alloc_tile_pool(name="wvpool", bufs=2)
    v = vp.tile([P, NBLK, D], BF16, tag="v")       # [m%128, mblock, d]
    wvqs = []
    for q4 in range(2):
        wvf = wvstage.tile([P, DC, 256], F32, tag="wvf")
        nc.scalar.dma_start(
            out=wvf,
            in_=wv[:, q4 * 256:(q4 + 1) * 256]
            .rearrange("(c p) f -> p c f", p=P))
        wvq = wvpool.tile([P, DC, 256], BF16, tag="wvq")
        nc.vector.tensor_copy(wvq, wvf)
        wvqs.append(wvq)

    # ---------------- phase 1c: qT = M^T x^T pass (all chunks resident) ---
    qstate = {}
    for w in range(NJ):
        qTj = p2q.tile([P, DC, 512], BF16, tag="qTj")
        qstate[w] = qTj
        for bc in range(DC):
            ps = ps4.tile([P, 512], F32, tag="ps4")
            for ic in range(DC):
                nc.tensor.matmul(
                    ps, msb[:, ic, bc * P:(bc + 1) * P],
                    xT[:, ic, w * 512:(w + 1) * 512],
                    start=(ic == 0), stop=(ic == DC - 1))
            copy_balanced(bc, qTj[:, bc, :], ps)
    msbp.release()

    # ---------------- phase 1d: v projection ----------------
    # natural [tok, d] layout: xT blocks stationary, Wv quarter-panels
    # (all 8 d_in chunks x 256 d_out) moving.
    for q4 in range(4):  # 256-wide d_out quarters
        if q4 < 2:
            wvq = wvqs[q4]
        else:
            wvf = wvstage.tile([P, DC, 256], F32, tag="wvf")
            nc.scalar.dma_start(
                out=wvf,
                in_=wv[:, q4 * 256:(q4 + 1) * 256]
                .rearrange("(c p) f -> p c f", p=P))
            wvq = wvpool.tile([P, DC, 256], BF16, tag="wvq")
            nc.vector.tensor_copy(wvq, wvf)
        for t in range(NBLK):
            ps = ps4.tile([P, 512], F32, tag="ps4")
            for dc in range(DC):
                nc.tensor.matmul(
                    ps[:, 0:256], xT[:, dc, t * P:(t + 1) * P],
                    wvq[:, dc, :],
                    start=(dc == 0), stop=(dc == DC - 1))
            copy_balanced(q4 + t, v[:, t, q4 * 256:(q4 + 1) * 256],
                          ps[:, 0:256])
    wvpool.release()
    wvstage.release()

    # ---------------- phase 2: attention ----------------
    ps4.release()
    ps2.release()
    ps512 = tc.alloc_tile_pool(name="ps512", bufs=4, space="PSUM")
    psbig = tc.alloc_tile_pool(name="psbig", bufs=2, space="PSUM")
    pden = tc.alloc_tile_pool(name="pden", bufs=3)
    pexp = tc.alloc_tile_pool(name="pexp", bufs=4)
    ppt = tc.alloc_tile_pool(name="ppt", bufs=4)
    posb = tc.alloc_tile_pool(name="posb", bufs=2)

    # Work units (j, il, mc) flattened; software-pipelined so the PE
    # transposes+AV of unit u-1 are emitted after the scores of unit u —
    # the ACT exp of u-1 then hides behind u's score matmuls.  il runs
    # descending so the last unit of the kernel has the shortest epilogue.
    units = []
    for j in range(NJ):
        for il in (3, 2, 1, 0):
            for mc in range(j + 1):
                units.append((j, il, mc))

    state = {}  # per-(j,il) live tiles: Ops, dpart

    def emit_scores(u):
        j, il, mc = u
        qTj = qstate[j]
        key = (j, il)
        if key not in state:
            state[key] = (psbig.tile([P, D], F32, tag="psbig", name="Ops"),
                          pden.tile([P, 6], F32, tag="dpart", name="dpart"))
        Ops, dpart = state[key]
        diag = (mc == j)
        # width of the valid score region in this chunk; keep >= 256 so
        # the fp32r matmul stays at full rate
        wv_ = max((il + 1) * P, 256) if diag else 512
        sS = ps512.tile([P, 512], F32, tag="ps512")
        for dc in range(DC):
            nc.tensor.matmul(
                sS[:, 0:wv_], qTj[:, dc, il * P:(il + 1) * P],
                xT[:, dc, mc * 512:mc * 512 + wv_],
                start=(dc == 0), stop=(dc == DC - 1))
        if diag:
            s0 = (3 - il) * P
            nc.vector.tensor_add(sS[:, 0:wv_], sS[:, 0:wv_],
                                 maskA[:, s0:s0 + wv_])
        expP = pexp.tile([P, 512], BF16, tag="expP")
        nc.scalar.activation(expP[:, 0:wv_], sS[:, 0:wv_], EXP, scale=SCALE,
                             accum_out=dpart[:, mc:mc + 1])
        return expP

    def emit_av(u, expP):
        j, il, mc = u
        Ops, dpart = state[(j, il)]
        nb = il + 1 if mc == j else 4
        ptp = ps512.tile([P, 512], BF16, tag="ps512")
        for b in range(nb):
            nc.tensor.transpose(
                ptp[:, b * P:(b + 1) * P],
                expP[:, b * P:(b + 1) * P], id16)
        PT = ppt.tile([P, 512], BF16, tag="PT")
        copy_balanced(4 * u[0] + u[2], PT[:, 0:nb * P], ptp[:, 0:nb * P])
        for b in range(nb):
            mb = 4 * mc + b
            last = (mc == j and b == nb - 1)
            for hf in range(2):
                nc.tensor.matmul(
                    Ops[:, hf * 512:(hf + 1) * 512],
                    PT[:, b * P:(b + 1) * P],
                    v[:, mb, hf * 512:(hf + 1) * 512],
                    start=(mc == 0 and b == 0), stop=last)
        if last:  # finish query block i = 4j + il
            i = 4 * j + il
            den = pden.tile([P, 2], F32, tag="den")
            nc.vector.reduce_sum(out=den[:, 0:1], in_=dpart[:, 0:j + 1],
                                 axis=AXX)
            nc.vector.reciprocal(den[:, 1:2], den[:, 0:1])
            Osb = posb.tile([P, D], F32, tag="Osb")
            for hf in range(2):
                nc.vector.tensor_scalar_mul(
                    Osb[:, hf * 512:(hf + 1) * 512],
                    Ops[:, hf * 512:(hf + 1) * 512], den[:, 1:2])
                nc.sync.dma_start(
                    out=out[i * P:(i + 1) * P, hf * 512:(hf + 1) * 512],
                    in_=Osb[:, hf * 512:(hf + 1) * 512])
            del state[(j, il)]

    prev = None  # (unit, expP)
    for u in units:
        expP = emit_scores(u)
        if prev is not None:
            emit_av(*prev)
        prev = (u, expP)
    emit_av(*prev)

    for pool in (posb, ppt, pexp, pden, vp, p2q, xTp, psbig, ps512, const):
        pool.release()


_NC_CACHE = None


def _get_nc():
    global _NC_CACHE
    if _NC_CACHE is None:
        _NC_CACHE = build_program()
    return _NC_CACHE


def kernel(x, W_query, W_key, W_value):
    """Full causal attention: x [8, 2048, 1024] -> [8, 2048, 1024] (f32)."""
    nc = _get_nc()
    x = np.ascontiguousarray(np.asarray(x, dtype=np.float32))
    wq = np.ascontiguousarray(np.asarray(W_query, dtype=np.float32))
    wk = np.ascontiguousarray(np.asarray(W_key, dtype=np.float32))
    wv = np.ascontiguousarray(np.asarray(W_value, dtype=np.float32))
    n_cores = x.shape[0]
    in_maps = [
        {"x": x[b], "W_query": wq, "W_key": wk, "W_value": wv}
        for b in range(n_cores)
    ]
    res = bass_utils.run_bass_kernel_spmd(nc, in_maps,
                                          core_ids=list(range(n_cores)))
    return np.stack([res.results[b]["out"] for b in range(n_cores)])


# revision 32
# speedup vs baseline: 1.0260x; 1.0260x over previous
"""Causal attention kernel for Trainium2 (Bass/Tile), 8-core data-parallel.

Problem: x [8, 2048, 1024] f32; W_query/W_key/W_value [1024, 1024] f32.
    q = x @ Wq; k = x @ Wk; v = x @ Wv       (per batch element)
    out = softmax(causal(q k^T) / 32) @ v

Sharding: batch dim (8) across the 8 NeuronCores, one batch element per
core; each core runs the identical single-core program on its slice.

Algorithm (per core): scores are computed as  S = x (Wq Wk^T) x^T,
which removes one full [2048,1024]x[1024,1024] projection pass: with
M = Wq Wk^T precomputed ([1024,1024], 64K PE-cycles incl. weight
transposes vs 128K for the k-pass), the "keys" operand of the score
matmuls is x^T itself, which is already resident for the projections.

Phase 1:
  Wq, Wk panels stream in; PE-transpose to WqT/WkT while accumulating
  the first 3 row-chunks of M in PSUM (pipelined against the DMA).
  Remaining 5 M chunks follow, then x streams in and is PE-transposed
  to xT [d_in, tok] (full 2048 tokens resident).
  qT = M^T x^T pass: chunks j=0,1 (tokens 0..1023) are written straight
  into the phase-2 SBUF tiles (no DRAM roundtrip); j=2,3 spill to DRAM.
  v = x Wv pass in natural [tok, d] layout (xT stationary, Wv moving).
Phase 2 (per 512-query chunk j, 128-query block i, descending il):
  S tiles via lhsT=qT block, rhs=xT chunks (causal: skip m > n chunks)
  additive causal mask on the diagonal chunk
  expP = exp(S/32) with fused row-sum (denominator partials)
  PE-transpose expP 128x128 blocks -> PT[m, n]
  O[n, d] += PT^T v  accumulated in PSUM over all valid m blocks
  O * (1/denominator) -> DMA out

All matmuls run in float32r (full PE rate for moving dim >= 256).
"""

import os

import numpy as np

# Defensive: recover wedged cores at NRT/PJRT init (no-op on healthy devices).
os.environ.setdefault("NEURON_RT_RESET_CORES", "1")

import concourse.tile as tile
import concourse.mybir as mybir
from concourse import bacc, bass_utils
from concourse.masks import make_identity

F32 = mybir.dt.float32
F32R = mybir.dt.float32r
BF16 = mybir.dt.bfloat16
EXP = mybir.ActivationFunctionType.Exp
AXX = mybir.AxisListType.X

NTOK = 2048      # tokens per batch element (= per core)
D = 1024         # d_in = d_out
P = 128          # partitions
DC = D // P      # 8 d-chunks
NBLK = NTOK // P     # 16 token blocks
NJ = NTOK // 512     # 4 query chunks of 512
NEG = -1.0e9
SCALE = 1.0 / 32.0   # 1/sqrt(D)
MA = 3           # M row-chunks accumulated during the weight stream


def build_program():
    nc = bacc.Bacc("TRN2", target_bir_lowering=False, debug=False,
                   num_devices=8)
    x = nc.dram_tensor("x", [NTOK, D], F32, kind="ExternalInput").ap()
    wq = nc.dram_tensor("W_query", [D, D], F32, kind="ExternalInput").ap()
    wk = nc.dram_tensor("W_key", [D, D], F32, kind="ExternalInput").ap()
    wv = nc.dram_tensor("W_value", [D, D], F32, kind="ExternalInput").ap()
    out = nc.dram_tensor("out", [NTOK, D], F32, kind="ExternalOutput").ap()

    with tile.TileContext(nc) as tc:
        _emit(nc, tc, x, wq, wk, wv, out)
    nc.compile()
    return nc


def _emit(nc, tc, x, wq, wk, wv, out):
    const = tc.alloc_tile_pool(name="const", bufs=1)

    # constants: identity for PE transpose, additive causal mask strip
    id32 = const.tile([P, P], F32, tag="id32")
    make_identity(nc, id32)
    id32r = const.tile([P, P], F32R, tag="id32r")
    nc.vector.tensor_copy(id32r, id32)
    id16 = const.tile([P, P], BF16, tag="id16")
    nc.vector.tensor_copy(id16, id32)
    # maskA = [0 x384 | causal(128) | NEG x384]; slice width 512 starting at
    # (3-il)*128 puts the causal block at in-chunk block position il.
    maskA = const.tile([P, 896], F32, tag="maskA")
    nc.vector.memset(maskA[:, 0:384], 0.0)
    nc.vector.memset(maskA[:, 384:512], 0.0)
    nc.gpsimd.affine_select(
        out=maskA[:, 384:512], in_=maskA[:, 384:512],
        compare_op=mybir.AluOpType.is_ge, fill=NEG, base=0,
        pattern=[[-1, P]], channel_multiplier=1)
    nc.vector.memset(maskA[:, 512:896], NEG)

    def copy_balanced(sel, out_ap, in_ap):
        # split PSUM->SBUF copy traffic across ACT and DVE
        if sel % 2 == 0:
            nc.scalar.copy(out_ap, in_ap)
        else:
            nc.vector.tensor_copy(out_ap, in_ap)

    # ---------------- phase 1a: weight transposes + M = Wq Wk^T ----------
    ps2 = tc.alloc_tile_pool(name="ps2", bufs=2, space="PSUM")   # 2 banks
    psM = tc.alloc_tile_pool(name="psM", bufs=3, space="PSUM")   # 6 banks
    msbp = tc.alloc_tile_pool(name="msbp", bufs=1, side="right")
    xnat = tc.alloc_tile_pool(name="xnat", bufs=32, side="right")
    xTp = tc.alloc_tile_pool(name="xTp", bufs=1)
    wqTp = tc.alloc_tile_pool(name="wqTp", bufs=1)
    wkTp = tc.alloc_tile_pool(name="wkTp", bufs=1)
    wstage = tc.alloc_tile_pool(name="wstage", bufs=6)

    xT = xTp.tile([P, DC, NTOK], BF16, tag="xT")  # [din%128, dinchunk, tok]
    wqT = wqTp.tile([P, DC, D], BF16, tag="wqT")   # [dout%128, oc, din]
    wkT = wkTp.tile([P, DC, D], BF16, tag="wkT")
    msb = msbp.tile([P, DC, D], BF16, tag="msb")   # M: [a%128, a-chunk, b]

    # DMA order: two x tiles (to fill the PE pipeline at t=0), then all W
    # panels (Wq/Wk alternating), then the rest of the x stream.
    xts = []

    def dma_x(tb, g):
        xt = xnat.tile([P, 512], F32R, tag="xt")
        nc.sync.dma_start(
            out=xt,
            in_=x[tb * P:(tb + 1) * P, g * 512:(g + 1) * 512].bitcast(F32R))
        xts.append(xt)

    def emit_xT(t):
        for g in range(2):
            xt = xts[t * 2 + g]
            trp = ps2.tile([P, 512], F32R, tag="ps2")
            for b4 in range(4):
                nc.tensor.transpose(
                    trp[:, b4 * P:(b4 + 1) * P],
                    xt[:, b4 * P:(b4 + 1) * P], id32r)
            copy_balanced(
                g, xT[:, g * 4:(g + 1) * 4, t * P:(t + 1) * P],
                trp.rearrange("p (b f) -> p b f", b=4))

    dma_x(0, 0)
    dma_x(0, 1)
    pans = []
    for oc in range(DC):
        for w_ap in (wq, wk):
            pan = wstage.tile([P, DC, P], F32R, tag="pan")
            nc.sync.dma_start(
                out=pan,
                in_=w_ap[:, oc * P:(oc + 1) * P]
                .rearrange("(c p) f -> p c f", p=P).bitcast(F32R))
            pans.append(pan)
    for tb in range(NBLK):
        for g in range(2):
            if tb == 0:
                continue
            dma_x(tb, g)
    emit_xT(0)  # PE starts here while the first W panel is in flight

    # W-T interleaved with the first MA chunks of M (PSUM-resident
    # accumulators, one matmul chain per chunk across all oc panels).
    Mps = [psM.tile([P, D], F32, tag="psM", name=f"Mps{a}") for a in range(MA)]
    for oc in range(DC):
        for iw, wT in ((0, wqT), (1, wkT)):
            pan = pans[oc * 2 + iw]
            for half in range(2):
                trp = ps2.tile([P, 512], F32R, tag="ps2")
                for ic4 in range(4):
                    ic = half * 4 + ic4
                    nc.tensor.transpose(
                        trp[:, ic4 * P:(ic4 + 1) * P], pan[:, ic, :], id32r)
                copy_balanced(oc + iw + half,
                              wT[:, oc, half * 512:(half + 1) * 512], trp)
        for a in range(MA):
            for h in range(2):  # PSUM bank limit: <=512 f32 out per matmul
                nc.tensor.matmul(
                    Mps[a][:, h * 512:(h + 1) * 512],
                    wqT[:, oc, a * P:(a + 1) * P],
                    wkT[:, oc, h * 512:(h + 1) * 512],
                    start=(oc == 0), stop=(oc == DC - 1))
    for a in range(MA):
        copy_balanced(a, msb[:, a, :], Mps[a])
    for a in range(MA, DC):
        ps = psM.tile([P, D], F32, tag="psM")
        for oc in range(DC):
            for h in range(2):
                nc.tensor.matmul(
                    ps[:, h * 512:(h + 1) * 512],
                    wqT[:, oc, a * P:(a + 1) * P],
                    wkT[:, oc, h * 512:(h + 1) * 512],
                    start=(oc == 0), stop=(oc == DC - 1))
        copy_balanced(a, msb[:, a, :], ps)

    wstage.release()
    wkTp.release()
    wqTp.release()
    psM.release()

    # ---------------- phase 1b: x transpose (t=0 was done up front) ------
    ps4 = tc.alloc_tile_pool(name="ps4", bufs=4, space="PSUM")
    for t in range(1, NBLK):
        emit_xT(t)
    xnat.release()

    # phase-2 / v-pass pools come up now so their first loads can issue on
    # the ACT queue early.
    p2q = tc.alloc_tile_pool(name="p2q", bufs=4)
    vp = tc.alloc_tile_pool(name="vp", bufs=1)
    wvstage = tc.alloc_tile_pool(name="wvstage", bufs=2)
    wvpool = tc.alloc_tile_pool(name="wvpool", bufs=2)
    v = vp.tile([P, NBLK, D], BF16, tag="v")       # [m%128, mblock, d]
    wvqs = []
    for q4 in range(2):
        wvf = wvstage.tile([P, DC, 256], F32, tag="wvf")
        nc.sync.dma_start(
            out=wvf,
            in_=wv[:, q4 * 256:(q4 + 1) * 256]
            .rearrange("(c p) f -> p c f", p=P))
        wvq = wvpool.tile([P, DC, 256], BF16, tag="wvq")
        nc.vector.tensor_copy(wvq, wvf)
        wvqs.append(wvq)

    # ---------------- phase 1c: qT = M^T x^T pass (all chunks resident) ---
    qstate = {}
    for w in range(NJ):
        qTj = p2q.tile([P, DC, 512], BF16, tag="qTj")
        qstate[w] = qTj
        for bc in range(DC):
            ps = ps4.tile([P, 512], F32, tag="ps4")
            for ic in range(DC):
                nc.tensor.matmul(
                    ps, msb[:, ic, bc * P:(bc + 1) * P],
                    xT[:, ic, w * 512:(w + 1) * 512],
                    start=(ic == 0), stop=(ic == DC - 1))
            copy_balanced(bc, qTj[:, bc, :], ps)
    msbp.release()

    # ---------------- phase 1d: v projection ----------------
    # natural [tok, d] layout: xT blocks stationary, Wv quarter-panels
    # (all 8 d_in chunks x 256 d_out) moving.
    for q4 in range(4):  # 256-wide d_out quarters
        if q4 < 2:
            wvq = wvqs[q4]
        else:
            wvf = wvstage.tile([P, DC, 256], F32, tag="wvf")
            nc.sync.dma_start(
                out=wvf,
                in_=wv[:, q4 * 256:(q4 + 1) * 256]
                .rearrange("(c p) f -> p c f", p=P))
            wvq = wvpool.tile([P, DC, 256], BF16, tag="wvq")
            nc.vector.tensor_copy(wvq, wvf)
        for t in range(NBLK):
            ps = ps4.tile([P, 512], F32, tag="ps4")
            for dc in range(DC):
                nc.tensor.matmul(
                    ps[:, 0:256], xT[:, dc, t * P:(t + 1) * P],
                    wvq[:, dc, :],
                    start=(dc == 0), stop=(dc == DC - 1))
            copy_balanced(q4 + t, v[:, t, q4 * 256:(q4 + 1) * 256],
                          ps[:, 0:256])
    wvpool.release()
    wvstage.release()

    # ---------------- phase 2: attention ----------------
    ps4.release()
    ps2.release()
    ps512 = tc.alloc_tile_pool(name="ps512", bufs=4, space="PSUM")
    psbig = tc.alloc_tile_pool(name="psbig", bufs=2, space="PSUM")
    pden = tc.alloc_tile_pool(name="pden", bufs=3)
    pexp = tc.alloc_tile_pool(name="pexp", bufs=4)
    ppt = tc.alloc_tile_pool(name="ppt", bufs=4)
    posb = tc.alloc_tile_pool(name="posb", bufs=2)

    # Work units (j, il, mc) flattened; software-pipelined so the PE
    # transposes+AV of unit u-1 are emitted after the scores of unit u —
    # the ACT exp of u-1 then hides behind u's score matmuls.  il runs
    # descending so the last unit of the kernel has the shortest epilogue.
    units = []
    for j in range(NJ):
        for il in (3, 2, 1, 0):
            for mc in range(j + 1):
                units.append((j, il, mc))

    state = {}  # per-(j,il) live tiles: Ops, dpart

    def emit_scores(u):
        j, il, mc = u
        qTj = qstate[j]
        key = (j, il)
        if key not in state:
            state[key] = (psbig.tile([P, D], F32, tag="psbig", name="Ops"),
                          pden.tile([P, 6], F32, tag="dpart", name="dpart"))
        Ops, dpart = state[key]
        diag = (mc == j)
        # width of the valid score region in this chunk; keep >= 256 so
        # the fp32r matmul stays at full rate
        wv_ = max((il + 1) * P, 256) if diag else 512
        sS = ps512.tile([P, 512], F32, tag="ps512")
        for dc in range(DC):
            nc.tensor.matmul(
                sS[:, 0:wv_], qTj[:, dc, il * P:(il + 1) * P],
                xT[:, dc, mc * 512:mc * 512 + wv_],
                start=(dc == 0), stop=(dc == DC - 1))
        if diag:
            s0 = (3 - il) * P
            nc.vector.tensor_add(sS[:, 0:wv_], sS[:, 0:wv_],
                                 maskA[:, s0:s0 + wv_])
        expP = pexp.tile([P, 512], BF16, tag="expP")
        nc.scalar.activation(expP[:, 0:wv_], sS[:, 0:wv_], EXP, scale=SCALE,
                             accum_out=dpart[:, mc:mc + 1])
        return expP

    def emit_av(u, expP):
        j, il, mc = u
        Ops, dpart = state[(j, il)]
        nb = il + 1 if mc == j else 4
        ptp = ps512.tile([P, 512], BF16, tag="ps512")
        for b in range(nb):
            nc.tensor.transpose(
                ptp[:, b * P:(b + 1) * P],
                expP[:, b * P:(b + 1) * P], id16)
        PT = ppt.tile([P, 512], BF16, tag="PT")
        copy_balanced(4 * u[0] + u[2], PT[:, 0:nb * P], ptp[:, 0:nb * P])
        for b in range(nb):
            mb = 4 * mc + b
            last = (mc == j and b == nb - 1)
            for hf in range(2):
                nc.tensor.matmul(
                    Ops[:, hf * 512:(hf + 1) * 512],
                    PT[:, b * P:(b + 1) * P],
                    v[:, mb, hf * 512:(hf + 1) * 512],
                    start=(mc == 0 and b == 0), stop=last)
        if last:  # finish query block i = 4j + il
            i = 4 * j + il
            den = pden.tile([P, 2], F32, tag="den")
            nc.vector.reduce_sum(out=den[:, 0:1], in_=dpart[:, 0:j + 1],
                                 axis=AXX)
            nc.vector.reciprocal(den[:, 1:2], den[:, 0:1])
            Osb = posb.tile([P, D], F32, tag="Osb")
            for hf in range(2):
                nc.vector.tensor_scalar_mul(
                    Osb[:, hf * 512:(hf + 1) * 512],
                    Ops[:, hf * 512:(hf + 1) * 512], den[:, 1:2])
                nc.sync.dma_start(
                    out=out[i * P:(i + 1) * P, hf * 512:(hf + 1) * 512],
                    in_=Osb[:, hf * 512:(hf + 1) * 512])
            del state[(j, il)]

    prev = None  # (unit, expP)
    for u in units:
        expP = emit_scores(u)
        if prev is not None:
            emit_av(*prev)
        prev = (u, expP)
    emit_av(*prev)

    for pool in (posb, ppt, pexp, pden, vp, p2q, xTp, psbig, ps512, const):
        pool.release()


_NC_CACHE = None


def _get_nc():
    global _NC_CACHE
    if _NC_CACHE is None:
        _NC_CACHE = build_program()
    return _NC_CACHE


def kernel(x, W_query, W_key, W_value):
    """Full causal attention: x [8, 2048, 1024] -> [8, 2048, 1024] (f32)."""
    nc = _get_nc()
    x = np.ascontiguousarray(np.asarray(x, dtype=np.float32))
    wq = np.ascontiguousarray(np.asarray(W_query, dtype=np.float32))
    wk = np.ascontiguousarray(np.asarray(W_key, dtype=np.float32))
    wv = np.ascontiguousarray(np.asarray(W_value, dtype=np.float32))
    n_cores = x.shape[0]
    in_maps = [
        {"x": x[b], "W_query": wq, "W_key": wk, "W_value": wv}
        for b in range(n_cores)
    ]
    res = bass_utils.run_bass_kernel_spmd(nc, in_maps,
                                          core_ids=list(range(n_cores)))
    return np.stack([res.results[b]["out"] for b in range(n_cores)])
